# revision 13
# baseline (speedup 1.0000x reference)
"""Trainium2 Bass kernel for a pre-norm transformer block (attention + GELU MLP).

Sharding (8 NeuronCores): core c handles batch b=c//4, rank r=c%4.
  - Attention is head-parallel: each core computes Q/K/V and softmax(QK^T)V for
    its 4 heads over the full 2048-token sequence of its batch.
  - One 8-core AllGather exchanges the per-head attention outputs (bf16,
    1 MB/rank); afterwards each core owns 512 tokens (quarter r of its batch)
    for the O-projection, LayerNorm2 and the full FFN — exactly 1/8 of the
    total FLOPs per core, with a single cheap collective.

On-chip activations stay feature-major (features on partitions, tokens on the
free axis) so no transposes are needed; LayerNorm statistics are computed with
ones-vector matmuls over the partition axis, softmax skips max-subtraction
(|scores| < ~3 by construction) and row-sums come from an appended ones column
on V. All reciprocals/rsqrts run as exp(p*ln(x)) on the scalar engine (the DVE
iterative divide costs ~4 us per 512-wide row); ln/exp share one activation
table set with the softmax exp, so no table switches happen until the GELU.
Matmuls run in bf16 with fp32 PSUM accumulation; softmax exp is evaluated on
1024-wide PSUM tiles to amortize the ACT fixed overhead.

The reference model's LayerNorm affines are identity (gamma=1, beta=0) and all
linear biases are zero — `kernel()` verifies this at runtime and dispatches to
a program that skips them (`trivial=True`); a general program handles the
non-trivial case.
"""

import sys

try:
    import concourse.bass as bass
except ImportError:
    sys.path.insert(0, "/opt/trn_rl_repo")
    import concourse.bass as bass

import numpy as np
import ml_dtypes

import concourse.bacc as bacc
import concourse.mybir as mybir
from concourse import tile
from concourse.bass_utils import run_bass_kernel_spmd

BF16 = mybir.dt.bfloat16
F32 = mybir.dt.float32
AF = mybir.ActivationFunctionType
OP = mybir.AluOpType

B, L, C, FF = 2, 2048, 1024, 4096
H, HD = 16, 64
NCORE, R = 8, 4          # 8 cores, 4 ranks per batch group
NH = H // R              # 4 heads per core
HDL = NH * HD            # 256 local head dims per core
T = L // R               # 512 tokens per core after the AllGather
SCALE = 1.0 / 8.0        # 1/sqrt(HD)
EPS = 1e-5
KC = C // 128            # 8 contraction chunks over C
FT = FF // 128           # 32 f-tiles
CT = C // 128            # 8 c-tiles
TCH = L // 512           # 4 token chunks of 512 (full sequence)
JT = L // 128            # 16 key tiles

# bias_pack column layout (fp32, each column is a per-partition [128,1] slice):
COL_BQ, COL_BK, COL_BO, COL_G1, COL_B1 = 0, 2, 4, 12, 20
COL_G2, COL_B2, COL_BF2, COL_BF1 = 28, 36, 44, 52
N_BIAS_COLS = 84

_prog_cache = {}


def build_program(trivial=True):
    nc = bacc.Bacc(None, target_bir_lowering=False, debug=False)

    xt = nc.declare_dram_parameter("xt", [C, L], BF16, isOutput=False)
    xres = nc.declare_dram_parameter("xres", [C, T], F32, isOutput=False)
    wqt = nc.declare_dram_parameter("wqt", [2, 128, KC, 128], BF16, isOutput=False)
    wkt = nc.declare_dram_parameter("wkt", [2, 128, KC, 128], BF16, isOutput=False)
    wvt = nc.declare_dram_parameter("wvt", [128, KC, HDL], BF16, isOutput=False)
    wot = nc.declare_dram_parameter("wot", [CT, 128, KC, 128], BF16, isOutput=False)
    w1t = nc.declare_dram_parameter("w1t", [FT, 128, KC, 128], BF16, isOutput=False)
    w2t = nc.declare_dram_parameter("w2t", [CT, 128, FT, 128], BF16, isOutput=False)
    biasp = nc.declare_dram_parameter("biasp", [128, N_BIAS_COLS], F32, isOutput=False)
    bvrow = nc.declare_dram_parameter("bvrow", [1, HDL], F32, isOutput=False)
    out = nc.declare_dram_parameter("out", [C, T], F32, isOutput=True)

    with tile.TileContext(nc) as tc:
        with (
            tc.tile_pool(name="const", bufs=1) as constp,
            tc.tile_pool(name="rows", bufs=1) as rows,
            tc.tile_pool(name="bcast", bufs=2) as bcp,
            tc.tile_pool(name="dram", bufs=1, space="DRAM") as dram,
        ):
            # ---- constants ----
            bias_sb = constp.tile([128, N_BIAS_COLS], F32)
            nc.sync.dma_start(out=bias_sb[:], in_=biasp[:])
            bv_sb = constp.tile([1, HDL], F32)
            nc.sync.dma_start(out=bv_sb[:], in_=bvrow[:])
            ones_bf = constp.tile([128, 1], BF16)
            nc.vector.memset(ones_bf[:], 1.0)
            if not trivial:
                bv_bc = constp.tile([128, HDL], F32)
                nc.gpsimd.partition_broadcast(bv_bc[:], bv_sb[:])

            def bias_ap(col):
                return bias_sb[:, col:col + 1]

            qt = [None, None]
            kt = [None, None]
            vbuf = []
            sends = [dram.tile([HDL, L // 2], BF16, tag=f"send{j}",
                               name=f"send{j}") for j in range(2)]

            ctx_qv = tc.tile_pool(name="qkt", bufs=1)
            qktp = ctx_qv.__enter__()
            ctx_vb = tc.tile_pool(name="vbufp", bufs=1)
            vbufp = ctx_vb.__enter__()

            # ================= phase A: LN1 + QKV + V =================
            with (
                tc.tile_pool(name="xtp", bufs=1) as xtp,
                tc.tile_pool(name="htp", bufs=1) as htp,
                tc.tile_pool(name="qkvw", bufs=1) as qkvw,
                tc.tile_pool(name="statps", bufs=1, space="PSUM") as statps,
                tc.tile_pool(name="mmps", bufs=3, space="PSUM") as mmps,
                tc.tile_pool(name="vps", bufs=2, space="PSUM") as vps,
            ):
                # ---- load x^T (bf16) ----
                xts = []
                for k in range(KC):
                    t_ = xtp.tile([128, L], BF16, tag=f"xt{k}", name=f"xt{k}")
                    nc.sync.dma_start(out=t_[:], in_=xt[k * 128:(k + 1) * 128, :])
                    xts.append(t_)

                # ---- LN1 stats per 512-chunk; rsig/mu*rsig rows in bf16 ----
                rsig_row = rows.tile([1, L], F32, bufs=1)
                murs_row = rows.tile([1, L], F32, bufs=1)
                with nc.named_scope("ln1"):
                    for tch in range(TCH):
                        sl = slice(tch * 512, (tch + 1) * 512)
                        pmu = statps.tile([1, 512], F32, tag="stat", name=f"pmu{tch}")
                        psq = statps.tile([1, 512], F32, tag="stat2", name=f"psq{tch}")
                        for k in range(KC):
                            nc.tensor.matmul(pmu[:], ones_bf[:], xts[k][:, sl],
                                             start=(k == 0), stop=(k == KC - 1))
                        for k in range(KC):
                            xs = bcp.tile([128, 512], BF16, tag="xsq", name=f"xs{tch}_{k}")
                            nc.vector.tensor_mul(xs[:], xts[k][:, sl], xts[k][:, sl])
                            nc.tensor.matmul(psq[:], ones_bf[:], xs[:],
                                             start=(k == 0), stop=(k == KC - 1))
                        mur = rows.tile([1, 512], F32, tag="mur", name=f"mur{tch}")
                        nc.scalar.activation(mur[:], pmu[:], AF.Copy, scale=1.0 / C)
                        msq = rows.tile([1, 512], F32, tag="msq", name=f"msq{tch}")
                        nc.scalar.activation(msq[:], psq[:], AF.Copy, scale=1.0 / C)
                        mu2 = rows.tile([1, 512], F32, tag="mu2", name=f"mu2_{tch}")
                        nc.vector.tensor_mul(mu2[:], mur[:], mur[:])
                        var = rows.tile([1, 512], F32, tag="var", name=f"var{tch}")
                        nc.vector.tensor_sub(var[:], msq[:], mu2[:])
                        nc.vector.tensor_scalar_add(var[:], var[:], EPS)
                        # rsig = exp(-0.5 * ln(var)) straight into the bf16 row
                        lnv = rows.tile([1, 512], F32, tag="lnv", name=f"lnv{tch}")
                        nc.scalar.activation(lnv[:], var[:], AF.Ln)
                        nc.scalar.activation(rsig_row[:, sl], lnv[:], AF.Exp,
                                             scale=-0.5)
                        nc.vector.tensor_mul(murs_row[:, sl], mur[:],
                                             rsig_row[:, sl])

                    # broadcast the full stat rows once (bf16, 2048 wide)
                    rb1 = bcp.tile([128, L], F32, tag="rb1", bufs=1)
                    nc.gpsimd.partition_broadcast(rb1[:], rsig_row[:])
                    mb1 = bcp.tile([128, L], F32, tag="mb1", bufs=1)
                    nc.gpsimd.partition_broadcast(mb1[:], murs_row[:])

                    # h^T = (x^T - mu)*rsig in bf16
                    hts = []
                    for k in range(KC):
                        ht = htp.tile([128, L], BF16, tag=f"ht{k}", name=f"ht{k}")
                        for tch in range(TCH):
                            sl = slice(tch * 512, (tch + 1) * 512)
                            tmp = bcp.tile([128, 512], F32, tag="lntmp",
                                           name=f"lt{k}_{tch}")
                            nc.vector.tensor_mul(tmp[:], xts[k][:, sl],
                                                 rb1[:, sl])
                            if trivial:
                                nc.vector.tensor_sub(ht[:, sl], tmp[:],
                                                     mb1[:, sl])
                            else:
                                tmp2 = bcp.tile([128, 512], F32, tag="lntmp2",
                                                name=f"lt2{k}_{tch}")
                                nc.vector.tensor_sub(tmp2[:], tmp[:], mb1[:, sl])
                                nc.scalar.activation(ht[:, sl], tmp2[:],
                                                     AF.Identity,
                                                     bias=bias_ap(COL_B1 + k),
                                                     scale=bias_ap(COL_G1 + k))
                        hts.append(ht)

                # ---- QKV projections ----
                with nc.named_scope("qkv"):
                    wq_sb, wk_sb = [], []
                    for m in range(2):
                        wq = qkvw.tile([128, KC, 128], BF16, tag=f"wq{m}", name=f"wq{m}")
                        nc.sync.dma_start(out=wq[:], in_=wqt[m])
                        wq_sb.append(wq)
                        wk = qkvw.tile([128, KC, 128], BF16, tag=f"wk{m}", name=f"wk{m}")
                        nc.sync.dma_start(out=wk[:], in_=wkt[m])
                        wk_sb.append(wk)
                    wv_sb = qkvw.tile([128, KC, HDL], BF16, tag="wv")
                    nc.sync.dma_start(out=wv_sb[:], in_=wvt[:])

                    for m in range(2):
                        qtile = qktp.tile([128, L], BF16, tag=f"qt{m}", name=f"qt{m}")
                        ktile = qktp.tile([128, L], BF16, tag=f"kt{m}", name=f"kt{m}")
                        for tch in range(TCH):
                            sl = slice(tch * 512, (tch + 1) * 512)
                            pq = mmps.tile([128, 512], F32, tag="mm", name=f"pq{m}_{tch}")
                            for k in range(KC):
                                nc.tensor.matmul(pq[:], wq_sb[m][:, k, :],
                                                 hts[k][:, sl],
                                                 start=(k == 0), stop=(k == KC - 1))
                            if trivial:
                                nc.scalar.activation(qtile[:, sl], pq[:], AF.Copy)
                            else:
                                nc.scalar.activation(qtile[:, sl], pq[:], AF.Identity,
                                                     bias=bias_ap(COL_BQ + m))
                            pk = mmps.tile([128, 512], F32, tag="mm", name=f"pk{m}_{tch}")
                            for k in range(KC):
                                nc.tensor.matmul(pk[:], wk_sb[m][:, k, :],
                                                 hts[k][:, sl],
                                                 start=(k == 0), stop=(k == KC - 1))
                            if trivial:
                                nc.scalar.activation(ktile[:, sl], pk[:], AF.Copy)
                            else:
                                nc.scalar.activation(ktile[:, sl], pk[:], AF.Identity,
                                                     bias=bias_ap(COL_BK + m))
                        qt[m] = qtile
                        kt[m] = ktile

                    # V in token-major layout with a ones column per head:
                    # vbuf[tt] is [128 tokens, NH, HD+1]
                    for tt in range(JT):
                        vb = vbufp.tile([128, NH, HD + 1], BF16, tag=f"vb{tt}",
                                        name=f"vb{tt}")
                        pv = vps.tile([128, HDL], F32, tag="v", name=f"pv{tt}")
                        for k in range(KC):
                            nc.tensor.matmul(pv[:],
                                             hts[k][:, tt * 128:(tt + 1) * 128],
                                             wv_sb[:, k, :],
                                             start=(k == 0), stop=(k == KC - 1))
                        pv_v = pv[:].rearrange("p (nh hd) -> p nh hd", nh=NH)
                        if trivial:
                            nc.vector.tensor_copy(vb[:, :, 0:HD], pv_v)
                        else:
                            bv_v = bv_bc[:].rearrange("p (nh hd) -> p nh hd", nh=NH)
                            nc.vector.tensor_add(vb[:, :, 0:HD], pv_v, bv_v)
                        nc.vector.memset(vb[:, :, HD:HD + 1], 1.0)
                        vbuf.append(vb)

            # ================= phase B + C: two token halves, pipelined =====
            # Attention runs half-by-half (queries [0:1024) then [1024:2048));
            # each half ends in its own AllGather, after which this core owns a
            # 256-token slab (cols 256*r of the half) for Wo/LN2/FFN. Tile's
            # scheduler fills attention's ACT-bound PE gaps with the previous
            # half's FFN matmuls.
            recvs = [dram.tile([NCORE * HDL, L // 2], BF16, tag=f"recv{j}",
                               name=f"recv{j}") for j in range(2)]

            pid = nc.gpsimd.partition_id()
            row0 = nc.gpsimd.snap((pid // 4) * (R * HDL))
            col0 = nc.gpsimd.snap((pid % 4) * (T // 2))

            with (
                tc.tile_pool(name="ep", bufs=3) as ep,
                tc.tile_pool(name="obfp", bufs=2) as obfp,
                tc.tile_pool(name="sps", bufs=2, space="PSUM") as sps,
                tc.tile_pool(name="pop", bufs=2, space="PSUM") as pop,
                tc.tile_pool(name="ofull", bufs=1) as ofp,
                tc.tile_pool(name="y1p", bufs=1) as y1p,
                tc.tile_pool(name="h2p", bufs=1) as h2p,
                tc.tile_pool(name="gp", bufs=1) as gp,
                tc.tile_pool(name="wst", bufs=3) as wst,
                tc.tile_pool(name="tmpp", bufs=2) as tmpp,
                tc.tile_pool(name="mm2", bufs=2, space="PSUM") as mm2,
            ):
                TH = T // 2   # 256-token slab per core per half

                def emit_attn_half(j):
                    q0 = j * 1024
                    with nc.named_scope(f"attn{j}"):
                        for h in range(NH):
                            m, off = h // 2, 64 * (h % 2)
                            po_ab = [
                                pop.tile([HD + 1, 512], F32, tag="po",
                                         name=f"po{j}_{h}_{hf}")
                                for hf in range(2)
                            ]
                            for jt in range(JT):
                                ps = sps.tile([128, 1024], F32, tag="s",
                                              name=f"ps{j}_{h}_{jt}")
                                for hf in range(2):
                                    nc.tensor.matmul(
                                        ps[:, hf * 512:(hf + 1) * 512],
                                        kt[m][off:off + 64,
                                              jt * 128:(jt + 1) * 128],
                                        qt[m][off:off + 64,
                                              q0 + hf * 512:q0 + (hf + 1) * 512],
                                        start=True, stop=True)
                                e = ep.tile([128, 1024], BF16, tag="e",
                                            name=f"e{j}_{h}_{jt}")
                                nc.scalar.activation(e[:], ps[:], AF.Exp,
                                                     scale=SCALE)
                                for hf in range(2):
                                    nc.tensor.matmul(
                                        po_ab[hf][:], vbuf[jt][:, h, :],
                                        e[:, hf * 512:(hf + 1) * 512],
                                        start=(jt == 0), stop=(jt == JT - 1))
                            for hf in range(2):
                                po = po_ab[hf]
                                sl = slice(hf * 512, (hf + 1) * 512)
                                lnr = rows.tile([1, 512], F32, bufs=2, tag="lnrec",
                                                name=f"lnr{j}_{h}_{hf}")
                                nc.scalar.activation(lnr[:], po[HD:HD + 1, :],
                                                     AF.Ln)
                                rec = rows.tile([1, 512], F32, bufs=2, tag="rec",
                                                name=f"rec{j}_{h}_{hf}")
                                nc.scalar.activation(rec[:], lnr[:], AF.Exp,
                                                     scale=-1.0)
                                rb = bcp.tile([64, 512], F32, tag="recb",
                                              name=f"rb{j}_{h}_{hf}")
                                nc.gpsimd.partition_broadcast(rb[:], rec[:])
                                ob = obfp.tile([64, 512], BF16, tag="ob",
                                               name=f"ob{j}_{h}_{hf}")
                                nc.vector.tensor_mul(ob[:], po[0:HD, :], rb[:])
                                nc.gpsimd.dma_start(
                                    out=sends[j][h * HD:(h + 1) * HD, sl],
                                    in_=ob[:])

                def emit_c_half(j):
                    """Wo + LN2 + FFN for this core's 256-token slab of half j."""
                    osl = slice(j * TH, (j + 1) * TH)   # cols in out/xres
                    ofull = []
                    for k in range(CT):
                        o_ = ofp.tile([128, TH], BF16, tag=f"of{k}",
                                      name=f"of{j}_{k}")
                        nc.gpsimd.dma_start(
                            out=o_[:],
                            in_=recvs[j][bass.ds(row0 + k * 128, 128),
                                         bass.ds(col0, TH)])
                        ofull.append(o_)

                    y1s = []
                    with nc.named_scope(f"wo{j}"):
                        for mtile in range(CT):
                            wo_sb = wst.tile([128, KC, 128], BF16, tag="wo",
                                             name=f"wo{j}_{mtile}")
                            nc.sync.dma_start(out=wo_sb[:], in_=wot[mtile])
                            py = mm2.tile([128, TH], F32, tag="mm",
                                          name=f"pyo{j}_{mtile}")
                            for k in range(KC):
                                nc.tensor.matmul(py[:], wo_sb[:, k, :],
                                                 ofull[k][:],
                                                 start=(k == 0),
                                                 stop=(k == KC - 1))
                            xr = tmpp.tile([128, TH], F32, tag="xr",
                                           name=f"xr{j}_{mtile}")
                            nc.sync.dma_start(
                                out=xr[:],
                                in_=xres[mtile * 128:(mtile + 1) * 128, osl])
                            y1 = y1p.tile([128, TH], F32, tag=f"y1{mtile}",
                                          name=f"y1{j}_{mtile}")
                            if trivial:
                                nc.vector.tensor_add(y1[:], py[:], xr[:])
                            else:
                                t_ = tmpp.tile([128, TH], F32, tag="wot",
                                               name=f"wt{j}_{mtile}")
                                nc.scalar.activation(t_[:], py[:], AF.Identity,
                                                     bias=bias_ap(COL_BO + mtile))
                                nc.vector.tensor_add(y1[:], t_[:], xr[:])
                            y1s.append(y1)

                    with nc.named_scope(f"ln2_{j}"):
                        ybs = []
                        pmu = mm2.tile([1, TH], F32, tag="mm", name=f"pmu2_{j}")
                        for k in range(CT):
                            yb = h2p.tile([128, TH], BF16, tag=f"yb{k}",
                                          name=f"yb{j}_{k}")
                            nc.vector.tensor_copy(yb[:], y1s[k][:])
                            ybs.append(yb)
                            nc.tensor.matmul(pmu[:], ones_bf[:], yb[:],
                                             start=(k == 0), stop=(k == CT - 1))
                        psq = mm2.tile([1, TH], F32, tag="mm", name=f"psq2_{j}")
                        for k in range(CT):
                            ys = tmpp.tile([128, TH], BF16, tag="ysq",
                                           name=f"ys{j}_{k}")
                            nc.vector.tensor_mul(ys[:], ybs[k][:], ybs[k][:])
                            nc.tensor.matmul(psq[:], ones_bf[:], ys[:],
                                             start=(k == 0), stop=(k == CT - 1))
                        mur = rows.tile([1, TH], F32, tag="mur", name=f"murl2_{j}")
                        nc.scalar.activation(mur[:], pmu[:], AF.Copy, scale=1.0 / C)
                        msq = rows.tile([1, TH], F32, tag="msq", name=f"msql2_{j}")
                        nc.scalar.activation(msq[:], psq[:], AF.Copy, scale=1.0 / C)
                        mu2 = rows.tile([1, TH], F32, tag="mu2", name=f"mu2l2_{j}")
                        nc.vector.tensor_mul(mu2[:], mur[:], mur[:])
                        var = rows.tile([1, TH], F32, tag="var", name=f"varl2_{j}")
                        nc.vector.tensor_sub(var[:], msq[:], mu2[:])
                        nc.vector.tensor_scalar_add(var[:], var[:], EPS)
                        lnv = rows.tile([1, TH], F32, tag="lnv", name=f"lnvl2_{j}")
                        nc.scalar.activation(lnv[:], var[:], AF.Ln)
                        rsig2 = rows.tile([1, TH], F32, tag="rsig2",
                                          name=f"rsig2_{j}")
                        nc.scalar.activation(rsig2[:], lnv[:], AF.Exp, scale=-0.5)
                        murs2 = rows.tile([1, TH], F32, tag="murs2",
                                          name=f"murs2_{j}")
                        nc.vector.tensor_mul(murs2[:], mur[:], rsig2[:])
                        rb2 = bcp.tile([128, TH], F32, tag="rsb2", name=f"rb2_{j}")
                        nc.gpsimd.partition_broadcast(rb2[:], rsig2[:])
                        mb2 = bcp.tile([128, TH], F32, tag="mrb2", name=f"mb2_{j}")
                        nc.gpsimd.partition_broadcast(mb2[:], murs2[:])

                        h2s = []
                        for k in range(CT):
                            t1 = tmpp.tile([128, TH], F32, tag="lntmp",
                                           name=f"l2t{j}_{k}")
                            nc.vector.tensor_mul(t1[:], ybs[k][:], rb2[:])
                            h2 = h2p.tile([128, TH], BF16, tag=f"h2{k}",
                                          name=f"h2_{j}_{k}")
                            if trivial:
                                nc.vector.tensor_sub(h2[:], t1[:], mb2[:])
                            else:
                                t2 = tmpp.tile([128, TH], BF16, tag="lntmp2",
                                               name=f"l2u{j}_{k}")
                                nc.vector.tensor_sub(t2[:], t1[:], mb2[:])
                                nc.scalar.activation(h2[:], t2[:], AF.Identity,
                                                     bias=bias_ap(COL_B2 + k),
                                                     scale=bias_ap(COL_G2 + k))
                            h2s.append(h2)

                    with nc.named_scope(f"ffn{j}"):
                        gts = []
                        for f in range(FT):
                            w1_sb = wst.tile([128, KC, 128], BF16, tag="w1",
                                             name=f"w1_{j}_{f}")
                            nc.sync.dma_start(out=w1_sb[:], in_=w1t[f])
                            pg = mm2.tile([128, TH], F32, tag="mm",
                                          name=f"pg{j}_{f}")
                            for k in range(KC):
                                nc.tensor.matmul(pg[:], w1_sb[:, k, :], h2s[k][:],
                                                 start=(k == 0),
                                                 stop=(k == KC - 1))
                            g = gp.tile([128, TH], BF16, tag=f"g{f}",
                                        name=f"g{j}_{f}")
                            if trivial:
                                nc.scalar.activation(g[:], pg[:], AF.Gelu)
                            else:
                                nc.scalar.activation(g[:], pg[:], AF.Gelu,
                                                     bias=bias_ap(COL_BF1 + f))
                            gts.append(g)

                        for mtile in range(CT):
                            w2_sb = wst.tile([128, FT, 128], BF16, tag="w2",
                                             name=f"w2_{j}_{mtile}")
                            nc.scalar.dma_start(out=w2_sb[:], in_=w2t[mtile])
                            py = mm2.tile([128, TH], F32, tag="mm",
                                          name=f"py2{j}_{mtile}")
                            for f in range(FT):
                                nc.tensor.matmul(py[:], w2_sb[:, f, :], gts[f][:],
                                                 start=(f == 0),
                                                 stop=(f == FT - 1))
                            yo = tmpp.tile([128, TH], F32, tag="yo",
                                           name=f"yo{j}_{mtile}")
                            if trivial:
                                nc.vector.tensor_add(yo[:], py[:], y1s[mtile][:])
                            else:
                                t_ = tmpp.tile([128, TH], F32, tag="y2t",
                                               name=f"zt{j}_{mtile}")
                                nc.scalar.activation(t_[:], py[:], AF.Identity,
                                                     bias=bias_ap(COL_BF2 + mtile))
                                nc.vector.tensor_add(yo[:], t_[:], y1s[mtile][:])
                            nc.sync.dma_start(
                                out=out[mtile * 128:(mtile + 1) * 128, osl],
                                in_=yo[:])

                def emit_ag(j):
                    with nc.named_scope(f"ag{j}"):
                        nc.gpsimd.collective_compute(
                            "AllGather", OP.bypass,
                            replica_groups=[list(range(NCORE))],
                            ins=[sends[j].opt()], outs=[recvs[j].opt()],
                        )

                emit_attn_half(0)
                emit_ag(0)
                emit_attn_half(1)
                emit_c_half(0)   # scheduler back-fills attn1's PE gaps
                emit_ag(1)
                emit_c_half(1)
            ctx_vb.__exit__(None, None, None)
            ctx_qv.__exit__(None, None, None)

    nc.finalize()
    return nc


def _prep_inputs(x, g1, b1, Wq, bq, Wk, bk, Wv, bv, Wo, bo, g2, b2, W1, bf1,
                 W2, bf2):
    bf = ml_dtypes.bfloat16
    f32 = np.float32

    def tile4(A, n_m):
        # A is [C, n_m*128] (already transposed): -> [n_m, 128, KC, 128]
        return np.ascontiguousarray(
            A.reshape(KC, 128, n_m, 128).transpose(2, 1, 0, 3)).astype(bf)

    w1t_full = np.ascontiguousarray(
        W1.T.reshape(KC, 128, FT, 128).transpose(2, 1, 0, 3)).astype(bf)
    w2t_full = np.ascontiguousarray(
        W2.T.reshape(FT, 128, CT, 128).transpose(2, 1, 0, 3)).astype(bf)
    wot_full = tile4(Wo.T, CT)

    trivial = (
        np.all(g1 == 1) and np.all(g2 == 1)
        and not (np.any(b1) or np.any(b2) or np.any(bq) or np.any(bk)
                 or np.any(bv) or np.any(bo) or np.any(bf1) or np.any(bf2))
    )

    in_maps = []
    for c in range(NCORE):
        b, r = divmod(c, R)
        hd0 = HDL * r
        xT = np.ascontiguousarray(x[b].T)

        pack = np.zeros((128, N_BIAS_COLS), f32)
        for j in range(2):
            pack[:, COL_BQ + j] = bq[hd0 + 128 * j: hd0 + 128 * (j + 1)]
            pack[:, COL_BK + j] = bk[hd0 + 128 * j: hd0 + 128 * (j + 1)]
        for j in range(CT):
            pack[:, COL_BO + j] = bo[128 * j: 128 * (j + 1)]
            pack[:, COL_G1 + j] = g1[128 * j: 128 * (j + 1)]
            pack[:, COL_B1 + j] = b1[128 * j: 128 * (j + 1)]
            pack[:, COL_G2 + j] = g2[128 * j: 128 * (j + 1)]
            pack[:, COL_B2 + j] = b2[128 * j: 128 * (j + 1)]
            pack[:, COL_BF2 + j] = bf2[128 * j: 128 * (j + 1)]
        for j in range(FT):
            pack[:, COL_BF1 + j] = bf1[128 * j: 128 * (j + 1)]

        # residual slab: this core owns tokens [1024*j + 256*r, +256) for j=0,1
        TH = T // 2
        xres_core = np.concatenate(
            [xT[:, 1024 * j + TH * r: 1024 * j + TH * (r + 1)] for j in range(2)],
            axis=1)
        in_maps.append({
            "xt": xT.astype(bf),
            "xres": np.ascontiguousarray(xres_core).astype(f32),
            "wqt": tile4(np.ascontiguousarray(Wq[hd0:hd0 + HDL, :].T), 2),
            "wkt": tile4(np.ascontiguousarray(Wk[hd0:hd0 + HDL, :].T), 2),
            "wvt": np.ascontiguousarray(
                Wv[hd0:hd0 + HDL, :].T.reshape(KC, 128, HDL)
                .transpose(1, 0, 2)).astype(bf),
            "wot": wot_full,
            "w1t": w1t_full,
            "w2t": w2t_full,
            "biasp": pack,
            "bvrow": np.ascontiguousarray(bv[hd0:hd0 + HDL]).reshape(1, HDL)
                .astype(f32),
        })
    return in_maps, trivial


def _run(in_maps, trivial=True, trace=False, trace_cores=None):
    key = f"nc_{trivial}"
    if key not in _prog_cache:
        _prog_cache[key] = build_program(trivial=trivial)
    nc = _prog_cache[key]
    return run_bass_kernel_spmd(
        nc, in_maps, core_ids=list(range(NCORE)), trace=trace,
        trace_cores=trace_cores)


def assemble_output(results):
    TH = T // 2
    out_full = np.empty((B, L, C), np.float32)
    for c in range(NCORE):
        b, r = divmod(c, R)
        y = results[c]["out"]
        for j in range(2):
            out_full[b, 1024 * j + TH * r: 1024 * j + TH * (r + 1), :] = \
                y[:, TH * j: TH * (j + 1)].T
    return out_full


def kernel(**inputs):
    in_maps, trivial = _prep_inputs(
        **{k: np.asarray(v) for k, v in inputs.items()})
    res = _run(in_maps, trivial=trivial)
    return assemble_output(res.results)


# revision 14
# speedup vs baseline: 1.1774x; 1.1774x over previous
"""Trainium2 Bass kernel for a pre-norm transformer block (attention + GELU MLP).

Sharding (8 NeuronCores): core c handles batch b=c//4, rank r=c%4.
  - Attention is head-parallel: each core computes Q/K/V and softmax(QK^T)V for
    its 4 heads over the full 2048-token sequence of its batch.
  - One 8-core AllGather exchanges the per-head attention outputs (bf16,
    1 MB/rank); afterwards each core owns 512 tokens (quarter r of its batch)
    for the O-projection, LayerNorm2 and the full FFN — exactly 1/8 of the
    total FLOPs per core, with a single cheap collective.

On-chip activations stay feature-major (features on partitions, tokens on the
free axis) so no transposes are needed; LayerNorm statistics are computed with
ones-vector matmuls over the partition axis, softmax skips max-subtraction
(|scores| < ~3 by construction) and row-sums come from an appended ones column
on V. All reciprocals/rsqrts run as exp(p*ln(x)) on the scalar engine (the DVE
iterative divide costs ~4 us per 512-wide row); ln/exp share one activation
table set with the softmax exp, so no table switches happen until the GELU.
Matmuls run in bf16 with fp32 PSUM accumulation; softmax exp is evaluated on
1024-wide PSUM tiles to amortize the ACT fixed overhead.

The reference model's LayerNorm affines are identity (gamma=1, beta=0) and all
linear biases are zero — `kernel()` verifies this at runtime and dispatches to
a program that skips them (`trivial=True`); a general program handles the
non-trivial case.
"""

import sys

try:
    import concourse.bass as bass
except ImportError:
    sys.path.insert(0, "/opt/trn_rl_repo")
    import concourse.bass as bass

import numpy as np
import ml_dtypes

import concourse.bacc as bacc
import concourse.mybir as mybir
from concourse import tile
from concourse.bass_utils import run_bass_kernel_spmd

BF16 = mybir.dt.bfloat16
F32 = mybir.dt.float32
AF = mybir.ActivationFunctionType
OP = mybir.AluOpType

B, L, C, FF = 2, 2048, 1024, 4096
H, HD = 16, 64
NCORE, R = 8, 4          # 8 cores, 4 ranks per batch group
NH = H // R              # 4 heads per core
HDL = NH * HD            # 256 local head dims per core
T = L // R               # 512 tokens per core after the AllGather
SCALE = 1.0 / 8.0        # 1/sqrt(HD)
EPS = 1e-5
KC = C // 128            # 8 contraction chunks over C
FT = FF // 128           # 32 f-tiles
CT = C // 128            # 8 c-tiles
TCH = L // 512           # 4 token chunks of 512 (full sequence)
JT = L // 128            # 16 key tiles

# bias_pack column layout (fp32, each column is a per-partition [128,1] slice):
COL_BQ, COL_BK, COL_BO, COL_G1, COL_B1 = 0, 2, 4, 12, 20
COL_G2, COL_B2, COL_BF2, COL_BF1 = 28, 36, 44, 52
N_BIAS_COLS = 84

_prog_cache = {}


def build_program(trivial=True):
    nc = bacc.Bacc(None, target_bir_lowering=False, debug=False)

    xt = nc.declare_dram_parameter("xt", [C, L], BF16, isOutput=False)
    xres = nc.declare_dram_parameter("xres", [C, T], F32, isOutput=False)
    wqt = nc.declare_dram_parameter("wqt", [2, 128, KC, 128], BF16, isOutput=False)
    wkt = nc.declare_dram_parameter("wkt", [2, 128, KC, 128], BF16, isOutput=False)
    wvt = nc.declare_dram_parameter("wvt", [128, KC, HDL], BF16, isOutput=False)
    wot = nc.declare_dram_parameter("wot", [CT, 128, KC, 128], BF16, isOutput=False)
    w1t = nc.declare_dram_parameter("w1t", [FT, 128, KC, 128], BF16, isOutput=False)
    w2t = nc.declare_dram_parameter("w2t", [CT, 128, FT, 128], BF16, isOutput=False)
    biasp = nc.declare_dram_parameter("biasp", [128, N_BIAS_COLS], F32, isOutput=False)
    bvrow = nc.declare_dram_parameter("bvrow", [1, HDL], F32, isOutput=False)
    out = nc.declare_dram_parameter("out", [C, T], F32, isOutput=True)

    with tile.TileContext(nc) as tc:
        with (
            tc.tile_pool(name="const", bufs=1) as constp,
            tc.tile_pool(name="rows", bufs=1) as rows,
            tc.tile_pool(name="bcast", bufs=2) as bcp,
            tc.tile_pool(name="dram", bufs=1, space="DRAM") as dram,
        ):
            # ---- constants ----
            bias_sb = constp.tile([128, N_BIAS_COLS], F32)
            nc.sync.dma_start(out=bias_sb[:], in_=biasp[:])
            bv_sb = constp.tile([1, HDL], F32)
            nc.sync.dma_start(out=bv_sb[:], in_=bvrow[:])
            ones_bf = constp.tile([128, 1], BF16)
            nc.vector.memset(ones_bf[:], 1.0)
            ones_bf2 = constp.tile([128, 1], BF16)
            nc.vector.memset(ones_bf2[:], 1.0)
            if not trivial:
                bv_bc = constp.tile([128, HDL], F32)
                nc.gpsimd.partition_broadcast(bv_bc[:], bv_sb[:])

            def bias_ap(col):
                return bias_sb[:, col:col + 1]

            qt = [None, None]
            kt = [None, None]
            kt2 = [None, None]
            vbuf = []
            vbuf2 = []
            sends = [dram.tile([HDL, L // 2], BF16, tag=f"send{j}",
                               name=f"send{j}") for j in range(2)]

            ctx_qv = tc.tile_pool(name="qkt", bufs=1)
            qktp = ctx_qv.__enter__()
            ctx_vb = tc.tile_pool(name="vbufp", bufs=1)
            vbufp = ctx_vb.__enter__()

            # ================= phase A: LN1 + QKV + V =================
            with (
                tc.tile_pool(name="xtp", bufs=1) as xtp,
                tc.tile_pool(name="htp", bufs=1) as htp,
                tc.tile_pool(name="qkvw", bufs=1) as qkvw,
                tc.tile_pool(name="statps", bufs=1, space="PSUM") as statps,
                tc.tile_pool(name="mmps", bufs=3, space="PSUM") as mmps,
                tc.tile_pool(name="vps", bufs=2, space="PSUM") as vps,
            ):
                # ---- load x^T (bf16) ----
                xts = []
                for k in range(KC):
                    t_ = xtp.tile([128, L], BF16, tag=f"xt{k}", name=f"xt{k}")
                    nc.sync.dma_start(out=t_[:], in_=xt[k * 128:(k + 1) * 128, :])
                    xts.append(t_)

                # ---- LN1 stats per 512-chunk; rsig/mu*rsig rows in bf16 ----
                rsig_row = rows.tile([1, L], F32, bufs=1)
                murs_row = rows.tile([1, L], F32, bufs=1)
                with nc.named_scope("ln1"):
                    for tch in range(TCH):
                        sl = slice(tch * 512, (tch + 1) * 512)
                        pmu = statps.tile([1, 512], F32, tag="stat", name=f"pmu{tch}")
                        psq = statps.tile([1, 512], F32, tag="stat2", name=f"psq{tch}")
                        for k in range(KC):
                            nc.tensor.matmul(pmu[:], ones_bf[:], xts[k][:, sl],
                                             start=(k == 0), stop=(k == KC - 1))
                            xs = bcp.tile([128, 512], BF16, tag="xsq", name=f"xs{tch}_{k}")
                            nc.vector.tensor_mul(xs[:], xts[k][:, sl], xts[k][:, sl])
                            nc.tensor.matmul(psq[:], ones_bf2[:], xs[:],
                                             start=(k == 0), stop=(k == KC - 1))
                        mur = rows.tile([1, 512], F32, tag="mur", name=f"mur{tch}")
                        nc.scalar.activation(mur[:], pmu[:], AF.Copy, scale=1.0 / C)
                        msq = rows.tile([1, 512], F32, tag="msq", name=f"msq{tch}")
                        nc.scalar.activation(msq[:], psq[:], AF.Copy, scale=1.0 / C)
                        mu2 = rows.tile([1, 512], F32, tag="mu2", name=f"mu2_{tch}")
                        nc.vector.tensor_mul(mu2[:], mur[:], mur[:])
                        var = rows.tile([1, 512], F32, tag="var", name=f"var{tch}")
                        nc.vector.tensor_sub(var[:], msq[:], mu2[:])
                        nc.vector.tensor_scalar_add(var[:], var[:], EPS)
                        # rsig = exp(-0.5 * ln(var)) straight into the bf16 row
                        lnv = rows.tile([1, 512], F32, tag="lnv", name=f"lnv{tch}")
                        nc.scalar.activation(lnv[:], var[:], AF.Ln)
                        nc.scalar.activation(rsig_row[:, sl], lnv[:], AF.Exp,
                                             scale=-0.5)
                        nc.vector.tensor_mul(murs_row[:, sl], mur[:],
                                             rsig_row[:, sl])

                    # broadcast the full stat rows once (bf16, 2048 wide)
                    rb1 = bcp.tile([128, L], F32, tag="rb1", bufs=1)
                    nc.gpsimd.partition_broadcast(rb1[:], rsig_row[:])
                    mb1 = bcp.tile([128, L], F32, tag="mb1", bufs=1)
                    nc.gpsimd.partition_broadcast(mb1[:], murs_row[:])

                    # h^T = (x^T - mu)*rsig in bf16
                    hts = []
                    for k in range(KC):
                        ht = htp.tile([128, L], BF16, tag=f"ht{k}", name=f"ht{k}")
                        for tch in range(TCH):
                            sl = slice(tch * 512, (tch + 1) * 512)
                            tmp = bcp.tile([128, 512], F32, tag="lntmp",
                                           name=f"lt{k}_{tch}")
                            nc.vector.tensor_mul(tmp[:], xts[k][:, sl],
                                                 rb1[:, sl])
                            if trivial:
                                nc.vector.tensor_sub(ht[:, sl], tmp[:],
                                                     mb1[:, sl])
                            else:
                                tmp2 = bcp.tile([128, 512], F32, tag="lntmp2",
                                                name=f"lt2{k}_{tch}")
                                nc.vector.tensor_sub(tmp2[:], tmp[:], mb1[:, sl])
                                nc.scalar.activation(ht[:, sl], tmp2[:],
                                                     AF.Identity,
                                                     bias=bias_ap(COL_B1 + k),
                                                     scale=bias_ap(COL_G1 + k))
                        hts.append(ht)

                # ---- QKV projections ----
                with nc.named_scope("qkv"):
                    wq_sb, wk_sb = [], []
                    for m in range(2):
                        wq = qkvw.tile([128, KC, 128], BF16, tag=f"wq{m}", name=f"wq{m}")
                        nc.sync.dma_start(out=wq[:], in_=wqt[m])
                        wq_sb.append(wq)
                        wk = qkvw.tile([128, KC, 128], BF16, tag=f"wk{m}", name=f"wk{m}")
                        nc.sync.dma_start(out=wk[:], in_=wkt[m])
                        wk_sb.append(wk)
                    wv_sb = qkvw.tile([128, KC, HDL], BF16, tag="wv")
                    nc.sync.dma_start(out=wv_sb[:], in_=wvt[:])

                    for m in range(2):
                        qtile = qktp.tile([128, L], BF16, tag=f"qt{m}", name=f"qt{m}")
                        ktile = qktp.tile([128, L], BF16, tag=f"kt{m}", name=f"kt{m}")
                        ktile2 = qktp.tile([128, L], BF16, tag=f"kt2{m}", name=f"kt2{m}")
                        for tch in range(TCH):
                            sl = slice(tch * 512, (tch + 1) * 512)
                            pq = mmps.tile([128, 512], F32, tag="mm", name=f"pq{m}_{tch}")
                            for k in range(KC):
                                nc.tensor.matmul(pq[:], wq_sb[m][:, k, :],
                                                 hts[k][:, sl],
                                                 start=(k == 0), stop=(k == KC - 1))
                            if trivial:
                                nc.scalar.activation(qtile[:, sl], pq[:], AF.Copy)
                            else:
                                nc.scalar.activation(qtile[:, sl], pq[:], AF.Identity,
                                                     bias=bias_ap(COL_BQ + m))
                            pk = mmps.tile([128, 512], F32, tag="mm", name=f"pk{m}_{tch}")
                            for k in range(KC):
                                nc.tensor.matmul(pk[:], wk_sb[m][:, k, :],
                                                 hts[k][:, sl],
                                                 start=(k == 0), stop=(k == KC - 1))
                            if trivial:
                                nc.scalar.activation(ktile[:, sl], pk[:], AF.Copy)
                            else:
                                nc.scalar.activation(ktile[:, sl], pk[:], AF.Identity,
                                                     bias=bias_ap(COL_BK + m))
                            nc.vector.tensor_copy(ktile2[:, sl], ktile[:, sl])
                        qt[m] = qtile
                        kt[m] = ktile
                        kt2[m] = ktile2

                    # V in token-major layout with a ones column per head:
                    # vbuf[tt] is [128 tokens, NH, HD+1]
                    for tt in range(JT):
                        vb = vbufp.tile([128, NH, HD + 1], BF16, tag=f"vb{tt}",
                                        name=f"vb{tt}")
                        pv = vps.tile([128, HDL], F32, tag="v", name=f"pv{tt}")
                        for k in range(KC):
                            nc.tensor.matmul(pv[:],
                                             hts[k][:, tt * 128:(tt + 1) * 128],
                                             wv_sb[:, k, :],
                                             start=(k == 0), stop=(k == KC - 1))
                        pv_v = pv[:].rearrange("p (nh hd) -> p nh hd", nh=NH)
                        if trivial:
                            nc.vector.tensor_copy(vb[:, :, 0:HD], pv_v)
                        else:
                            bv_v = bv_bc[:].rearrange("p (nh hd) -> p nh hd", nh=NH)
                            nc.vector.tensor_add(vb[:, :, 0:HD], pv_v, bv_v)
                        nc.vector.memset(vb[:, :, HD:HD + 1], 1.0)
                        vbuf.append(vb)
                        vb2 = vbufp.tile([128, NH, HD + 1], BF16, tag=f"vc{tt}",
                                         name=f"vc{tt}")
                        nc.vector.tensor_copy(vb2[:], vb[:])
                        vbuf2.append(vb2)

            # ================= phase B + C: two token halves, pipelined =====
            # Attention runs half-by-half (queries [0:1024) then [1024:2048));
            # each half ends in its own AllGather, after which this core owns a
            # 256-token slab (cols 256*r of the half) for Wo/LN2/FFN. Tile's
            # scheduler fills attention's ACT-bound PE gaps with the previous
            # half's FFN matmuls.
            recvs = [dram.tile([NCORE * HDL, L // 2], BF16, tag=f"recv{j}",
                               name=f"recv{j}") for j in range(2)]

            pid = nc.gpsimd.partition_id()
            row0 = nc.gpsimd.snap((pid // 4) * (R * HDL))
            col0 = nc.gpsimd.snap((pid % 4) * (T // 2))

            with (
                tc.tile_pool(name="ep", bufs=3) as ep,
                tc.tile_pool(name="obfp", bufs=2) as obfp,
                tc.tile_pool(name="sps", bufs=2, space="PSUM") as sps,
                tc.tile_pool(name="pop", bufs=2, space="PSUM") as pop,
                tc.tile_pool(name="ofull", bufs=1) as ofp,
                tc.tile_pool(name="y1p", bufs=1) as y1p,
                tc.tile_pool(name="h2p", bufs=1) as h2p,
                tc.tile_pool(name="gp", bufs=1) as gp,
                tc.tile_pool(name="wst", bufs=3) as wst,
                tc.tile_pool(name="tmpp", bufs=2) as tmpp,
                tc.tile_pool(name="mm2", bufs=2, space="PSUM") as mm2,
            ):
                TH = T // 2   # 256-token slab per core per half

                def emit_attn_half(j):
                    q0 = j * 1024
                    with nc.named_scope(f"attn{j}"):
                        for h in range(NH):
                            m, off = h // 2, 64 * (h % 2)
                            po_ab = [
                                pop.tile([HD + 1, 512], F32, tag="po",
                                         name=f"po{j}_{h}_{hf}")
                                for hf in range(2)
                            ]
                            for jt in range(JT):
                                ps = sps.tile([128, 1024], F32, tag="s",
                                              name=f"ps{j}_{h}_{jt}")
                                for hf in range(2):
                                    ksrc = kt[m] if hf == 0 else kt2[m]
                                    nc.tensor.matmul(
                                        ps[:, hf * 512:(hf + 1) * 512],
                                        ksrc[off:off + 64,
                                             jt * 128:(jt + 1) * 128],
                                        qt[m][off:off + 64,
                                              q0 + hf * 512:q0 + (hf + 1) * 512],
                                        start=True, stop=True)
                                e = ep.tile([128, 1024], BF16, tag="e",
                                            name=f"e{j}_{h}_{jt}")
                                nc.scalar.activation(e[:], ps[:], AF.Exp,
                                                     scale=SCALE)
                                for hf in range(2):
                                    vsrc = vbuf if hf == 0 else vbuf2
                                    nc.tensor.matmul(
                                        po_ab[hf][:], vsrc[jt][:, h, :],
                                        e[:, hf * 512:(hf + 1) * 512],
                                        start=(jt == 0), stop=(jt == JT - 1))
                            for hf in range(2):
                                po = po_ab[hf]
                                sl = slice(hf * 512, (hf + 1) * 512)
                                lnr = rows.tile([1, 512], F32, bufs=2, tag="lnrec",
                                                name=f"lnr{j}_{h}_{hf}")
                                nc.scalar.activation(lnr[:], po[HD:HD + 1, :],
                                                     AF.Ln)
                                rec = rows.tile([1, 512], F32, bufs=2, tag="rec",
                                                name=f"rec{j}_{h}_{hf}")
                                nc.scalar.activation(rec[:], lnr[:], AF.Exp,
                                                     scale=-1.0)
                                rb = bcp.tile([64, 512], F32, tag="recb",
                                              name=f"rb{j}_{h}_{hf}")
                                nc.gpsimd.partition_broadcast(rb[:], rec[:])
                                ob = obfp.tile([64, 512], BF16, tag="ob",
                                               name=f"ob{j}_{h}_{hf}")
                                nc.vector.tensor_mul(ob[:], po[0:HD, :], rb[:])
                                nc.gpsimd.dma_start(
                                    out=sends[j][h * HD:(h + 1) * HD, sl],
                                    in_=ob[:])

                def emit_c_half(j):
                    """Wo + LN2 + FFN for this core's 256-token slab of half j."""
                    osl = slice(j * TH, (j + 1) * TH)   # cols in out/xres
                    ofull = []
                    for k in range(CT):
                        o_ = ofp.tile([128, TH], BF16, tag=f"of{k}",
                                      name=f"of{j}_{k}")
                        nc.gpsimd.dma_start(
                            out=o_[:],
                            in_=recvs[j][bass.ds(row0 + k * 128, 128),
                                         bass.ds(col0, TH)])
                        ofull.append(o_)

                    y1s = []
                    with nc.named_scope(f"wo{j}"):
                        for mtile in range(CT):
                            wo_sb = wst.tile([128, KC, 128], BF16, tag="wo",
                                             name=f"wo{j}_{mtile}")
                            nc.sync.dma_start(out=wo_sb[:], in_=wot[mtile])
                            py = mm2.tile([128, TH], F32, tag="mm",
                                          name=f"pyo{j}_{mtile}")
                            for k in range(KC):
                                nc.tensor.matmul(py[:], wo_sb[:, k, :],
                                                 ofull[k][:],
                                                 start=(k == 0),
                                                 stop=(k == KC - 1))
                            xr = tmpp.tile([128, TH], F32, tag="xr",
                                           name=f"xr{j}_{mtile}")
                            nc.sync.dma_start(
                                out=xr[:],
                                in_=xres[mtile * 128:(mtile + 1) * 128, osl])
                            y1 = y1p.tile([128, TH], F32, tag=f"y1{mtile}",
                                          name=f"y1{j}_{mtile}")
                            if trivial:
                                nc.vector.tensor_add(y1[:], py[:], xr[:])
                            else:
                                t_ = tmpp.tile([128, TH], F32, tag="wot",
                                               name=f"wt{j}_{mtile}")
                                nc.scalar.activation(t_[:], py[:], AF.Identity,
                                                     bias=bias_ap(COL_BO + mtile))
                                nc.vector.tensor_add(y1[:], t_[:], xr[:])
                            y1s.append(y1)

                    with nc.named_scope(f"ln2_{j}"):
                        ybs = []
                        pmu = mm2.tile([1, TH], F32, tag="mm", name=f"pmu2_{j}")
                        psq = mm2.tile([1, TH], F32, tag="mm", name=f"psq2_{j}")
                        for k in range(CT):
                            yb = h2p.tile([128, TH], BF16, tag=f"yb{k}",
                                          name=f"yb{j}_{k}")
                            nc.vector.tensor_copy(yb[:], y1s[k][:])
                            ybs.append(yb)
                            nc.tensor.matmul(pmu[:], ones_bf[:], yb[:],
                                             start=(k == 0), stop=(k == CT - 1))
                            ys = tmpp.tile([128, TH], BF16, tag="ysq",
                                           name=f"ys{j}_{k}")
                            nc.vector.tensor_mul(ys[:], ybs[k][:], ybs[k][:])
                            nc.tensor.matmul(psq[:], ones_bf2[:], ys[:],
                                             start=(k == 0), stop=(k == CT - 1))
                        mur = rows.tile([1, TH], F32, tag="mur", name=f"murl2_{j}")
                        nc.scalar.activation(mur[:], pmu[:], AF.Copy, scale=1.0 / C)
                        msq = rows.tile([1, TH], F32, tag="msq", name=f"msql2_{j}")
                        nc.scalar.activation(msq[:], psq[:], AF.Copy, scale=1.0 / C)
                        mu2 = rows.tile([1, TH], F32, tag="mu2", name=f"mu2l2_{j}")
                        nc.vector.tensor_mul(mu2[:], mur[:], mur[:])
                        var = rows.tile([1, TH], F32, tag="var", name=f"varl2_{j}")
                        nc.vector.tensor_sub(var[:], msq[:], mu2[:])
                        nc.vector.tensor_scalar_add(var[:], var[:], EPS)
                        lnv = rows.tile([1, TH], F32, tag="lnv", name=f"lnvl2_{j}")
                        nc.scalar.activation(lnv[:], var[:], AF.Ln)
                        rsig2 = rows.tile([1, TH], F32, tag="rsig2",
                                          name=f"rsig2_{j}")
                        nc.scalar.activation(rsig2[:], lnv[:], AF.Exp, scale=-0.5)
                        murs2 = rows.tile([1, TH], F32, tag="murs2",
                                          name=f"murs2_{j}")
                        nc.vector.tensor_mul(murs2[:], mur[:], rsig2[:])
                        rb2 = bcp.tile([128, TH], F32, tag="rsb2", name=f"rb2_{j}")
                        nc.gpsimd.partition_broadcast(rb2[:], rsig2[:])
                        mb2 = bcp.tile([128, TH], F32, tag="mrb2", name=f"mb2_{j}")
                        nc.gpsimd.partition_broadcast(mb2[:], murs2[:])

                        h2s = []
                        for k in range(CT):
                            t1 = tmpp.tile([128, TH], F32, tag="lntmp",
                                           name=f"l2t{j}_{k}")
                            nc.vector.tensor_mul(t1[:], ybs[k][:], rb2[:])
                            h2 = h2p.tile([128, TH], BF16, tag=f"h2{k}",
                                          name=f"h2_{j}_{k}")
                            if trivial:
                                nc.vector.tensor_sub(h2[:], t1[:], mb2[:])
                            else:
                                t2 = tmpp.tile([128, TH], BF16, tag="lntmp2",
                                               name=f"l2u{j}_{k}")
                                nc.vector.tensor_sub(t2[:], t1[:], mb2[:])
                                nc.scalar.activation(h2[:], t2[:], AF.Identity,
                                                     bias=bias_ap(COL_B2 + k),
                                                     scale=bias_ap(COL_G2 + k))
                            h2s.append(h2)

                    with nc.named_scope(f"ffn{j}"):
                        gts = []
                        for f in range(FT):
                            w1_sb = wst.tile([128, KC, 128], BF16, tag="w1",
                                             name=f"w1_{j}_{f}")
                            nc.sync.dma_start(out=w1_sb[:], in_=w1t[f])
                            pg = mm2.tile([128, TH], F32, tag="mm",
                                          name=f"pg{j}_{f}")
                            for k in range(KC):
                                nc.tensor.matmul(pg[:], w1_sb[:, k, :], h2s[k][:],
                                                 start=(k == 0),
                                                 stop=(k == KC - 1))
                            g = gp.tile([128, TH], BF16, tag=f"g{f}",
                                        name=f"g{j}_{f}")
                            if trivial:
                                nc.scalar.activation(g[:], pg[:], AF.Gelu)
                            else:
                                nc.scalar.activation(g[:], pg[:], AF.Gelu,
                                                     bias=bias_ap(COL_BF1 + f))
                            gts.append(g)

                        for mtile in range(CT):
                            w2_sb = wst.tile([128, FT, 128], BF16, tag="w2",
                                             name=f"w2_{j}_{mtile}")
                            nc.scalar.dma_start(out=w2_sb[:], in_=w2t[mtile])
                            py = mm2.tile([128, TH], F32, tag="mm",
                                          name=f"py2{j}_{mtile}")
                            for f in range(FT):
                                nc.tensor.matmul(py[:], w2_sb[:, f, :], gts[f][:],
                                                 start=(f == 0),
                                                 stop=(f == FT - 1))
                            yo = tmpp.tile([128, TH], F32, tag="yo",
                                           name=f"yo{j}_{mtile}")
                            if trivial:
                                nc.vector.tensor_add(yo[:], py[:], y1s[mtile][:])
                            else:
                                t_ = tmpp.tile([128, TH], F32, tag="y2t",
                                               name=f"zt{j}_{mtile}")
                                nc.scalar.activation(t_[:], py[:], AF.Identity,
                                                     bias=bias_ap(COL_BF2 + mtile))
                                nc.vector.tensor_add(yo[:], t_[:], y1s[mtile][:])
                            nc.sync.dma_start(
                                out=out[mtile * 128:(mtile + 1) * 128, osl],
                                in_=yo[:])

                def emit_ag(j):
                    with nc.named_scope(f"ag{j}"):
                        nc.gpsimd.collective_compute(
                            "AllGather", OP.bypass,
                            replica_groups=[list(range(NCORE))],
                            ins=[sends[j].opt()], outs=[recvs[j].opt()],
                        )

                emit_attn_half(0)
                emit_ag(0)
                emit_attn_half(1)
                emit_c_half(0)   # scheduler back-fills attn1's PE gaps
                emit_ag(1)
                emit_c_half(1)
            ctx_vb.__exit__(None, None, None)
            ctx_qv.__exit__(None, None, None)

    nc.finalize()
    return nc


def _prep_inputs(x, g1, b1, Wq, bq, Wk, bk, Wv, bv, Wo, bo, g2, b2, W1, bf1,
                 W2, bf2):
    bf = ml_dtypes.bfloat16
    f32 = np.float32

    def tile4(A, n_m):
        # A is [C, n_m*128] (already transposed): -> [n_m, 128, KC, 128]
        return np.ascontiguousarray(
            A.reshape(KC, 128, n_m, 128).transpose(2, 1, 0, 3)).astype(bf)

    w1t_full = np.ascontiguousarray(
        W1.T.reshape(KC, 128, FT, 128).transpose(2, 1, 0, 3)).astype(bf)
    w2t_full = np.ascontiguousarray(
        W2.T.reshape(FT, 128, CT, 128).transpose(2, 1, 0, 3)).astype(bf)
    wot_full = tile4(Wo.T, CT)

    trivial = (
        np.all(g1 == 1) and np.all(g2 == 1)
        and not (np.any(b1) or np.any(b2) or np.any(bq) or np.any(bk)
                 or np.any(bv) or np.any(bo) or np.any(bf1) or np.any(bf2))
    )

    in_maps = []
    for c in range(NCORE):
        b, r = divmod(c, R)
        hd0 = HDL * r
        xT = np.ascontiguousarray(x[b].T)

        pack = np.zeros((128, N_BIAS_COLS), f32)
        for j in range(2):
            pack[:, COL_BQ + j] = bq[hd0 + 128 * j: hd0 + 128 * (j + 1)]
            pack[:, COL_BK + j] = bk[hd0 + 128 * j: hd0 + 128 * (j + 1)]
        for j in range(CT):
            pack[:, COL_BO + j] = bo[128 * j: 128 * (j + 1)]
            pack[:, COL_G1 + j] = g1[128 * j: 128 * (j + 1)]
            pack[:, COL_B1 + j] = b1[128 * j: 128 * (j + 1)]
            pack[:, COL_G2 + j] = g2[128 * j: 128 * (j + 1)]
            pack[:, COL_B2 + j] = b2[128 * j: 128 * (j + 1)]
            pack[:, COL_BF2 + j] = bf2[128 * j: 128 * (j + 1)]
        for j in range(FT):
            pack[:, COL_BF1 + j] = bf1[128 * j: 128 * (j + 1)]

        # residual slab: this core owns tokens [1024*j + 256*r, +256) for j=0,1
        TH = T // 2
        xres_core = np.concatenate(
            [xT[:, 1024 * j + TH * r: 1024 * j + TH * (r + 1)] for j in range(2)],
            axis=1)
        in_maps.append({
            "xt": xT.astype(bf),
            "xres": np.ascontiguousarray(xres_core).astype(f32),
            "wqt": tile4(np.ascontiguousarray(Wq[hd0:hd0 + HDL, :].T), 2),
            "wkt": tile4(np.ascontiguousarray(Wk[hd0:hd0 + HDL, :].T), 2),
            "wvt": np.ascontiguousarray(
                Wv[hd0:hd0 + HDL, :].T.reshape(KC, 128, HDL)
                .transpose(1, 0, 2)).astype(bf),
            "wot": wot_full,
            "w1t": w1t_full,
            "w2t": w2t_full,
            "biasp": pack,
            "bvrow": np.ascontiguousarray(bv[hd0:hd0 + HDL]).reshape(1, HDL)
                .astype(f32),
        })
    return in_maps, trivial


def _run(in_maps, trivial=True, trace=False, trace_cores=None):
    key = f"nc_{trivial}"
    if key not in _prog_cache:
        _prog_cache[key] = build_program(trivial=trivial)
    nc = _prog_cache[key]
    return run_bass_kernel_spmd(
        nc, in_maps, core_ids=list(range(NCORE)), trace=trace,
        trace_cores=trace_cores)


def assemble_output(results):
    TH = T // 2
    out_full = np.empty((B, L, C), np.float32)
    for c in range(NCORE):
        b, r = divmod(c, R)
        y = results[c]["out"]
        for j in range(2):
            out_full[b, 1024 * j + TH * r: 1024 * j + TH * (r + 1), :] = \
                y[:, TH * j: TH * (j + 1)].T
    return out_full


def kernel(**inputs):
    in_maps, trivial = _prep_inputs(
        **{k: np.asarray(v) for k, v in inputs.items()})
    res = _run(in_maps, trivial=trivial)
    return assemble_output(res.results)


# revision 16
# speedup vs baseline: 1.1817x; 1.0036x over previous
"""Trainium2 Bass kernel for a pre-norm transformer block (attention + GELU MLP).

Sharding (8 NeuronCores): core c handles batch b=c//4, rank r=c%4.
  - Attention is head-parallel: each core computes Q/K/V and softmax(QK^T)V for
    its 4 heads over the full 2048-token sequence of its batch.
  - One 8-core AllGather exchanges the per-head attention outputs (bf16,
    1 MB/rank); afterwards each core owns 512 tokens (quarter r of its batch)
    for the O-projection, LayerNorm2 and the full FFN — exactly 1/8 of the
    total FLOPs per core, with a single cheap collective.

On-chip activations stay feature-major (features on partitions, tokens on the
free axis) so no transposes are needed; LayerNorm statistics are computed with
ones-vector matmuls over the partition axis, softmax skips max-subtraction
(|scores| < ~3 by construction) and row-sums come from an appended ones column
on V. All reciprocals/rsqrts run as exp(p*ln(x)) on the scalar engine (the DVE
iterative divide costs ~4 us per 512-wide row); ln/exp share one activation
table set with the softmax exp, so no table switches happen until the GELU.
Matmuls run in bf16 with fp32 PSUM accumulation; softmax exp is evaluated on
1024-wide PSUM tiles to amortize the ACT fixed overhead.

The reference model's LayerNorm affines are identity (gamma=1, beta=0) and all
linear biases are zero — `kernel()` verifies this at runtime and dispatches to
a program that skips them (`trivial=True`); a general program handles the
non-trivial case.
"""

import sys

try:
    import concourse.bass as bass
except ImportError:
    sys.path.insert(0, "/opt/trn_rl_repo")
    import concourse.bass as bass

import numpy as np
import ml_dtypes

import concourse.bacc as bacc
import concourse.mybir as mybir
from concourse import tile
from concourse.bass_utils import run_bass_kernel_spmd

BF16 = mybir.dt.bfloat16
F32 = mybir.dt.float32
AF = mybir.ActivationFunctionType
OP = mybir.AluOpType

B, L, C, FF = 2, 2048, 1024, 4096
H, HD = 16, 64
NCORE, R = 8, 4          # 8 cores, 4 ranks per batch group
NH = H // R              # 4 heads per core
HDL = NH * HD            # 256 local head dims per core
T = L // R               # 512 tokens per core after the AllGather
SCALE = 1.0 / 8.0        # 1/sqrt(HD)
EPS = 1e-5
KC = C // 128            # 8 contraction chunks over C
FT = FF // 128           # 32 f-tiles
CT = C // 128            # 8 c-tiles
TCH = L // 512           # 4 token chunks of 512 (full sequence)
JT = L // 128            # 16 key tiles

# bias_pack column layout (fp32, each column is a per-partition [128,1] slice):
COL_BQ, COL_BK, COL_BO, COL_G1, COL_B1 = 0, 2, 4, 12, 20
COL_G2, COL_B2, COL_BF2, COL_BF1 = 28, 36, 44, 52
N_BIAS_COLS = 84

_prog_cache = {}


def build_program(trivial=True):
    nc = bacc.Bacc(None, target_bir_lowering=False, debug=False)

    xt = nc.declare_dram_parameter("xt", [C, L], BF16, isOutput=False)
    xres = nc.declare_dram_parameter("xres", [C, T], F32, isOutput=False)
    wqt = nc.declare_dram_parameter("wqt", [2, 128, KC, 128], BF16, isOutput=False)
    wkt = nc.declare_dram_parameter("wkt", [2, 128, KC, 128], BF16, isOutput=False)
    wvt = nc.declare_dram_parameter("wvt", [128, KC, HDL], BF16, isOutput=False)
    wot = nc.declare_dram_parameter("wot", [CT, 128, KC, 128], BF16, isOutput=False)
    w1t = nc.declare_dram_parameter("w1t", [FT, 128, KC, 128], BF16, isOutput=False)
    w2t = nc.declare_dram_parameter("w2t", [CT, 128, FT, 128], BF16, isOutput=False)
    biasp = nc.declare_dram_parameter("biasp", [128, N_BIAS_COLS], F32, isOutput=False)
    bvrow = nc.declare_dram_parameter("bvrow", [1, HDL], F32, isOutput=False)
    out = nc.declare_dram_parameter("out", [C, T], F32, isOutput=True)

    with tile.TileContext(nc) as tc:
        with (
            tc.tile_pool(name="const", bufs=1) as constp,
            tc.tile_pool(name="rows", bufs=1) as rows,
            tc.tile_pool(name="bcast", bufs=2) as bcp,
            tc.tile_pool(name="dram", bufs=1, space="DRAM") as dram,
        ):
            # ---- constants ----
            bias_sb = constp.tile([128, N_BIAS_COLS], F32)
            nc.sync.dma_start(out=bias_sb[:], in_=biasp[:])
            bv_sb = constp.tile([1, HDL], F32)
            nc.sync.dma_start(out=bv_sb[:], in_=bvrow[:])
            ones_bf = constp.tile([128, 1], BF16)
            nc.vector.memset(ones_bf[:], 1.0)
            ones_bf2 = constp.tile([128, 1], BF16)
            nc.vector.memset(ones_bf2[:], 1.0)
            if not trivial:
                bv_bc = constp.tile([128, HDL], F32)
                nc.gpsimd.partition_broadcast(bv_bc[:], bv_sb[:])

            def bias_ap(col):
                return bias_sb[:, col:col + 1]

            qt = [None, None]
            kt = [None, None]
            kt2 = [None, None]
            vbuf = []
            vbuf2 = []
            sends = [dram.tile([HDL, L // 2], BF16, tag=f"send{j}",
                               name=f"send{j}") for j in range(2)]

            ctx_qv = tc.tile_pool(name="qkt", bufs=1)
            qktp = ctx_qv.__enter__()
            ctx_vb = tc.tile_pool(name="vbufp", bufs=1)
            vbufp = ctx_vb.__enter__()

            # ================= phase A: LN1 + QKV + V =================
            with (
                tc.tile_pool(name="xtp", bufs=1) as xtp,
                tc.tile_pool(name="htp", bufs=1) as htp,
                tc.tile_pool(name="qkvw", bufs=1) as qkvw,
                tc.tile_pool(name="statps", bufs=1, space="PSUM") as statps,
                tc.tile_pool(name="mmps", bufs=3, space="PSUM") as mmps,
                tc.tile_pool(name="vps", bufs=2, space="PSUM") as vps,
            ):
                # ---- load x^T (bf16) ----
                xts = []
                for k in range(KC):
                    t_ = xtp.tile([128, L], BF16, tag=f"xt{k}", name=f"xt{k}")
                    nc.sync.dma_start(out=t_[:], in_=xt[k * 128:(k + 1) * 128, :])
                    xts.append(t_)

                # ---- LN1 stats per 512-chunk; rsig/mu*rsig rows in bf16 ----
                rsig_row = rows.tile([1, L], F32, bufs=1)
                murs_row = rows.tile([1, L], F32, bufs=1)
                with nc.named_scope("ln1"):
                    for tch in range(TCH):
                        sl = slice(tch * 512, (tch + 1) * 512)
                        pmu = statps.tile([1, 512], F32, tag="stat", name=f"pmu{tch}")
                        psq = statps.tile([1, 512], F32, tag="stat2", name=f"psq{tch}")
                        for k in range(KC):
                            nc.tensor.matmul(pmu[:], ones_bf[:], xts[k][:, sl],
                                             start=(k == 0), stop=(k == KC - 1))
                            xs = bcp.tile([128, 512], BF16, tag="xsq", name=f"xs{tch}_{k}")
                            nc.vector.tensor_mul(xs[:], xts[k][:, sl], xts[k][:, sl])
                            nc.tensor.matmul(psq[:], ones_bf2[:], xs[:],
                                             start=(k == 0), stop=(k == KC - 1))
                        mur = rows.tile([1, 512], F32, tag="mur", name=f"mur{tch}")
                        nc.scalar.activation(mur[:], pmu[:], AF.Copy, scale=1.0 / C)
                        msq = rows.tile([1, 512], F32, tag="msq", name=f"msq{tch}")
                        nc.scalar.activation(msq[:], psq[:], AF.Copy, scale=1.0 / C)
                        mu2 = rows.tile([1, 512], F32, tag="mu2", name=f"mu2_{tch}")
                        nc.vector.tensor_mul(mu2[:], mur[:], mur[:])
                        var = rows.tile([1, 512], F32, tag="var", name=f"var{tch}")
                        nc.vector.tensor_sub(var[:], msq[:], mu2[:])
                        nc.vector.tensor_scalar_add(var[:], var[:], EPS)
                        # rsig = exp(-0.5 * ln(var)) straight into the bf16 row
                        lnv = rows.tile([1, 512], F32, tag="lnv", name=f"lnv{tch}")
                        nc.scalar.activation(lnv[:], var[:], AF.Ln)
                        nc.scalar.activation(rsig_row[:, sl], lnv[:], AF.Exp,
                                             scale=-0.5)
                        nc.vector.tensor_mul(murs_row[:, sl], mur[:],
                                             rsig_row[:, sl])

                    # broadcast the full stat rows once (bf16, 2048 wide)
                    rb1 = bcp.tile([128, L], F32, tag="rb1", bufs=1)
                    nc.gpsimd.partition_broadcast(rb1[:], rsig_row[:])
                    mb1 = bcp.tile([128, L], F32, tag="mb1", bufs=1)
                    nc.gpsimd.partition_broadcast(mb1[:], murs_row[:])

                    # h^T = (x^T - mu)*rsig in bf16
                    hts = []
                    for k in range(KC):
                        ht = htp.tile([128, L], BF16, tag=f"ht{k}", name=f"ht{k}")
                        for tch in range(TCH):
                            sl = slice(tch * 512, (tch + 1) * 512)
                            tmp = bcp.tile([128, 512], F32, tag="lntmp",
                                           name=f"lt{k}_{tch}")
                            nc.vector.tensor_mul(tmp[:], xts[k][:, sl],
                                                 rb1[:, sl])
                            if trivial:
                                nc.vector.tensor_sub(ht[:, sl], tmp[:],
                                                     mb1[:, sl])
                            else:
                                tmp2 = bcp.tile([128, 512], F32, tag="lntmp2",
                                                name=f"lt2{k}_{tch}")
                                nc.vector.tensor_sub(tmp2[:], tmp[:], mb1[:, sl])
                                nc.scalar.activation(ht[:, sl], tmp2[:],
                                                     AF.Identity,
                                                     bias=bias_ap(COL_B1 + k),
                                                     scale=bias_ap(COL_G1 + k))
                        hts.append(ht)

                # ---- QKV projections ----
                with nc.named_scope("qkv"):
                    wq_sb, wk_sb = [], []
                    for m in range(2):
                        wq = qkvw.tile([128, KC, 128], BF16, tag=f"wq{m}", name=f"wq{m}")
                        nc.sync.dma_start(out=wq[:], in_=wqt[m])
                        wq_sb.append(wq)
                        wk = qkvw.tile([128, KC, 128], BF16, tag=f"wk{m}", name=f"wk{m}")
                        nc.sync.dma_start(out=wk[:], in_=wkt[m])
                        wk_sb.append(wk)
                    wv_sb = qkvw.tile([128, KC, HDL], BF16, tag="wv")
                    nc.sync.dma_start(out=wv_sb[:], in_=wvt[:])

                    for m in range(2):
                        qtile = qktp.tile([128, L], BF16, tag=f"qt{m}", name=f"qt{m}")
                        ktile = qktp.tile([128, L], BF16, tag=f"kt{m}", name=f"kt{m}")
                        ktile2 = qktp.tile([128, L], BF16, tag=f"kt2{m}", name=f"kt2{m}")
                        for tch in range(TCH):
                            sl = slice(tch * 512, (tch + 1) * 512)
                            pq = mmps.tile([128, 512], F32, tag="mm", name=f"pq{m}_{tch}")
                            for k in range(KC):
                                nc.tensor.matmul(pq[:], wq_sb[m][:, k, :],
                                                 hts[k][:, sl],
                                                 start=(k == 0), stop=(k == KC - 1))
                            if trivial:
                                nc.scalar.activation(qtile[:, sl], pq[:], AF.Copy)
                            else:
                                nc.scalar.activation(qtile[:, sl], pq[:], AF.Identity,
                                                     bias=bias_ap(COL_BQ + m))
                            pk = mmps.tile([128, 512], F32, tag="mm", name=f"pk{m}_{tch}")
                            for k in range(KC):
                                nc.tensor.matmul(pk[:], wk_sb[m][:, k, :],
                                                 hts[k][:, sl],
                                                 start=(k == 0), stop=(k == KC - 1))
                            if trivial:
                                nc.scalar.activation(ktile[:, sl], pk[:], AF.Copy)
                            else:
                                nc.scalar.activation(ktile[:, sl], pk[:], AF.Identity,
                                                     bias=bias_ap(COL_BK + m))
                            nc.vector.tensor_copy(ktile2[:, sl], ktile[:, sl])
                        qt[m] = qtile
                        kt[m] = ktile
                        kt2[m] = ktile2

                    # V in token-major layout with a ones column per head:
                    # vbuf[tt] is [128 tokens, NH, HD+1]
                    for tt in range(JT):
                        vb = vbufp.tile([128, NH, HD + 1], BF16, tag=f"vb{tt}",
                                        name=f"vb{tt}")
                        pv = vps.tile([128, HDL], F32, tag="v", name=f"pv{tt}")
                        for k in range(KC):
                            nc.tensor.matmul(pv[:],
                                             hts[k][:, tt * 128:(tt + 1) * 128],
                                             wv_sb[:, k, :],
                                             start=(k == 0), stop=(k == KC - 1))
                        pv_v = pv[:].rearrange("p (nh hd) -> p nh hd", nh=NH)
                        if trivial:
                            nc.vector.tensor_copy(vb[:, :, 0:HD], pv_v)
                        else:
                            bv_v = bv_bc[:].rearrange("p (nh hd) -> p nh hd", nh=NH)
                            nc.vector.tensor_add(vb[:, :, 0:HD], pv_v, bv_v)
                        nc.vector.memset(vb[:, :, HD:HD + 1], 1.0)
                        vbuf.append(vb)
                        vb2 = vbufp.tile([128, NH, HD + 1], BF16, tag=f"vc{tt}",
                                         name=f"vc{tt}")
                        nc.vector.tensor_copy(vb2[:], vb[:])
                        vbuf2.append(vb2)

            # ================= phase B + C: two token halves, pipelined =====
            # Attention runs half-by-half (queries [0:1024) then [1024:2048));
            # each half ends in its own AllGather, after which this core owns a
            # 256-token slab (cols 256*r of the half) for Wo/LN2/FFN. Tile's
            # scheduler fills attention's ACT-bound PE gaps with the previous
            # half's FFN matmuls.
            recvs = [dram.tile([NCORE * HDL, L // 2], BF16, tag=f"recv{j}",
                               name=f"recv{j}") for j in range(2)]

            pid = nc.gpsimd.partition_id()
            row0 = nc.gpsimd.snap((pid // 4) * (R * HDL))
            col0 = nc.gpsimd.snap((pid % 4) * (T // 2))

            with (
                tc.tile_pool(name="ep", bufs=3) as ep,
                tc.tile_pool(name="obfp", bufs=2) as obfp,
                tc.tile_pool(name="sps", bufs=2, space="PSUM") as sps,
                tc.tile_pool(name="pop", bufs=2, space="PSUM") as pop,
                tc.tile_pool(name="ofull", bufs=1) as ofp,
                tc.tile_pool(name="y1p", bufs=1) as y1p,
                tc.tile_pool(name="h2p", bufs=1) as h2p,
                tc.tile_pool(name="gp", bufs=1) as gp,
                tc.tile_pool(name="wst", bufs=3) as wst,
                tc.tile_pool(name="tmpp", bufs=2) as tmpp,
                tc.tile_pool(name="mm2", bufs=2, space="PSUM") as mm2,
            ):
                TH = T // 2   # 256-token slab per core per half

                def emit_attn_half(j):
                    q0 = j * 1024
                    with nc.named_scope(f"attn{j}"):
                        for h in range(NH):
                            m, off = h // 2, 64 * (h % 2)
                            po_ab = [
                                pop.tile([HD + 1, 512], F32, tag="po",
                                         name=f"po{j}_{h}_{hf}")
                                for hf in range(2)
                            ]
                            for jt in range(JT):
                                ps = sps.tile([128, 1024], F32, tag="s",
                                              name=f"ps{j}_{h}_{jt}")
                                for hf in range(2):
                                    ksrc = kt[m] if hf == 0 else kt2[m]
                                    nc.tensor.matmul(
                                        ps[:, hf * 512:(hf + 1) * 512],
                                        ksrc[off:off + 64,
                                             jt * 128:(jt + 1) * 128],
                                        qt[m][off:off + 64,
                                              q0 + hf * 512:q0 + (hf + 1) * 512],
                                        start=True, stop=True)
                                e = ep.tile([128, 1024], BF16, tag="e",
                                            name=f"e{j}_{h}_{jt}")
                                nc.scalar.activation(e[:], ps[:], AF.Exp,
                                                     scale=SCALE)
                                for hf in range(2):
                                    vsrc = vbuf if hf == 0 else vbuf2
                                    nc.tensor.matmul(
                                        po_ab[hf][:], vsrc[jt][:, h, :],
                                        e[:, hf * 512:(hf + 1) * 512],
                                        start=(jt == 0), stop=(jt == JT - 1))
                            for hf in range(2):
                                po = po_ab[hf]
                                sl = slice(hf * 512, (hf + 1) * 512)
                                obf = obfp.tile([HD + 1, 512], F32, tag="obf",
                                                name=f"obf{j}_{h}_{hf}")
                                nc.vector.tensor_copy(obf[:], po[:])
                                lnr = rows.tile([1, 512], F32, bufs=2, tag="lnrec",
                                                name=f"lnr{j}_{h}_{hf}")
                                nc.scalar.activation(lnr[:], obf[HD:HD + 1, :],
                                                     AF.Ln)
                                rec = rows.tile([1, 512], F32, bufs=2, tag="rec",
                                                name=f"rec{j}_{h}_{hf}")
                                nc.scalar.activation(rec[:], lnr[:], AF.Exp,
                                                     scale=-1.0)
                                rb = bcp.tile([HD, 512], F32, tag="recb",
                                              name=f"rb{j}_{h}_{hf}")
                                nc.gpsimd.partition_broadcast(rb[:], rec[:])
                                ob = obfp.tile([HD, 512], BF16, tag="ob",
                                               name=f"ob{j}_{h}_{hf}")
                                nc.vector.tensor_mul(ob[:], obf[0:HD, :], rb[:])
                                nc.gpsimd.dma_start(
                                    out=sends[j][h * HD:(h + 1) * HD, sl],
                                    in_=ob[:])

                def emit_c_half(j):
                    """Wo + LN2 + FFN for this core's 256-token slab of half j."""
                    osl = slice(j * TH, (j + 1) * TH)   # cols in out/xres
                    ofull = []
                    for k in range(CT):
                        o_ = ofp.tile([128, TH], BF16, tag=f"of{k}",
                                      name=f"of{j}_{k}")
                        nc.gpsimd.dma_start(
                            out=o_[:],
                            in_=recvs[j][bass.ds(row0 + k * 128, 128),
                                         bass.ds(col0, TH)])
                        ofull.append(o_)

                    y1s = []
                    with nc.named_scope(f"wo{j}"):
                        for mtile in range(CT):
                            wo_sb = wst.tile([128, KC, 128], BF16, tag="wo",
                                             name=f"wo{j}_{mtile}")
                            nc.sync.dma_start(out=wo_sb[:], in_=wot[mtile])
                            py = mm2.tile([128, TH], F32, tag="mm",
                                          name=f"pyo{j}_{mtile}")
                            for k in range(KC):
                                nc.tensor.matmul(py[:], wo_sb[:, k, :],
                                                 ofull[k][:],
                                                 start=(k == 0),
                                                 stop=(k == KC - 1))
                            xr = tmpp.tile([128, TH], F32, tag="xr",
                                           name=f"xr{j}_{mtile}")
                            nc.sync.dma_start(
                                out=xr[:],
                                in_=xres[mtile * 128:(mtile + 1) * 128, osl])
                            y1 = y1p.tile([128, TH], F32, tag=f"y1{mtile}",
                                          name=f"y1{j}_{mtile}")
                            if trivial:
                                nc.vector.tensor_add(y1[:], py[:], xr[:])
                            else:
                                t_ = tmpp.tile([128, TH], F32, tag="wot",
                                               name=f"wt{j}_{mtile}")
                                nc.scalar.activation(t_[:], py[:], AF.Identity,
                                                     bias=bias_ap(COL_BO + mtile))
                                nc.vector.tensor_add(y1[:], t_[:], xr[:])
                            y1s.append(y1)

                    with nc.named_scope(f"ln2_{j}"):
                        ybs = []
                        pmu = mm2.tile([1, TH], F32, tag="mm", name=f"pmu2_{j}")
                        psq = mm2.tile([1, TH], F32, tag="mm", name=f"psq2_{j}")
                        for k in range(CT):
                            yb = h2p.tile([128, TH], BF16, tag=f"yb{k}",
                                          name=f"yb{j}_{k}")
                            nc.vector.tensor_copy(yb[:], y1s[k][:])
                            ybs.append(yb)
                            nc.tensor.matmul(pmu[:], ones_bf[:], yb[:],
                                             start=(k == 0), stop=(k == CT - 1))
                            ys = tmpp.tile([128, TH], BF16, tag="ysq",
                                           name=f"ys{j}_{k}")
                            nc.vector.tensor_mul(ys[:], ybs[k][:], ybs[k][:])
                            nc.tensor.matmul(psq[:], ones_bf2[:], ys[:],
                                             start=(k == 0), stop=(k == CT - 1))
                        mur = rows.tile([1, TH], F32, tag="mur", name=f"murl2_{j}")
                        nc.scalar.activation(mur[:], pmu[:], AF.Copy, scale=1.0 / C)
                        msq = rows.tile([1, TH], F32, tag="msq", name=f"msql2_{j}")
                        nc.scalar.activation(msq[:], psq[:], AF.Copy, scale=1.0 / C)
                        mu2 = rows.tile([1, TH], F32, tag="mu2", name=f"mu2l2_{j}")
                        nc.vector.tensor_mul(mu2[:], mur[:], mur[:])
                        var = rows.tile([1, TH], F32, tag="var", name=f"varl2_{j}")
                        nc.vector.tensor_sub(var[:], msq[:], mu2[:])
                        nc.vector.tensor_scalar_add(var[:], var[:], EPS)
                        lnv = rows.tile([1, TH], F32, tag="lnv", name=f"lnvl2_{j}")
                        nc.scalar.activation(lnv[:], var[:], AF.Ln)
                        rsig2 = rows.tile([1, TH], F32, tag="rsig2",
                                          name=f"rsig2_{j}")
                        nc.scalar.activation(rsig2[:], lnv[:], AF.Exp, scale=-0.5)
                        murs2 = rows.tile([1, TH], F32, tag="murs2",
                                          name=f"murs2_{j}")
                        nc.vector.tensor_mul(murs2[:], mur[:], rsig2[:])
                        rb2 = bcp.tile([128, TH], F32, tag="rsb2", name=f"rb2_{j}")
                        nc.gpsimd.partition_broadcast(rb2[:], rsig2[:])
                        mb2 = bcp.tile([128, TH], F32, tag="mrb2", name=f"mb2_{j}")
                        nc.gpsimd.partition_broadcast(mb2[:], murs2[:])

                        h2s = []
                        for k in range(CT):
                            t1 = tmpp.tile([128, TH], F32, tag="lntmp",
                                           name=f"l2t{j}_{k}")
                            nc.vector.tensor_mul(t1[:], ybs[k][:], rb2[:])
                            h2 = h2p.tile([128, TH], BF16, tag=f"h2{k}",
                                          name=f"h2_{j}_{k}")
                            if trivial:
                                nc.vector.tensor_sub(h2[:], t1[:], mb2[:])
                            else:
                                t2 = tmpp.tile([128, TH], BF16, tag="lntmp2",
                                               name=f"l2u{j}_{k}")
                                nc.vector.tensor_sub(t2[:], t1[:], mb2[:])
                                nc.scalar.activation(h2[:], t2[:], AF.Identity,
                                                     bias=bias_ap(COL_B2 + k),
                                                     scale=bias_ap(COL_G2 + k))
                            h2s.append(h2)

                    with nc.named_scope(f"ffn{j}"):
                        gts = []
                        for f in range(FT):
                            w1_sb = wst.tile([128, KC, 128], BF16, tag="w1",
                                             name=f"w1_{j}_{f}")
                            nc.sync.dma_start(out=w1_sb[:], in_=w1t[f])
                            pg = mm2.tile([128, TH], F32, tag="mm",
                                          name=f"pg{j}_{f}")
                            for k in range(KC):
                                nc.tensor.matmul(pg[:], w1_sb[:, k, :], h2s[k][:],
                                                 start=(k == 0),
                                                 stop=(k == KC - 1))
                            g = gp.tile([128, TH], BF16, tag=f"g{f}",
                                        name=f"g{j}_{f}")
                            if trivial:
                                nc.scalar.activation(g[:], pg[:], AF.Gelu)
                            else:
                                nc.scalar.activation(g[:], pg[:], AF.Gelu,
                                                     bias=bias_ap(COL_BF1 + f))
                            gts.append(g)

                        for mtile in range(CT):
                            w2_sb = wst.tile([128, FT, 128], BF16, tag="w2",
                                             name=f"w2_{j}_{mtile}")
                            nc.scalar.dma_start(out=w2_sb[:], in_=w2t[mtile])
                            py = mm2.tile([128, TH], F32, tag="mm",
                                          name=f"py2{j}_{mtile}")
                            for f in range(FT):
                                nc.tensor.matmul(py[:], w2_sb[:, f, :], gts[f][:],
                                                 start=(f == 0),
                                                 stop=(f == FT - 1))
                            yo = tmpp.tile([128, TH], F32, tag="yo",
                                           name=f"yo{j}_{mtile}")
                            if trivial:
                                nc.vector.tensor_add(yo[:], py[:], y1s[mtile][:])
                            else:
                                t_ = tmpp.tile([128, TH], F32, tag="y2t",
                                               name=f"zt{j}_{mtile}")
                                nc.scalar.activation(t_[:], py[:], AF.Identity,
                                                     bias=bias_ap(COL_BF2 + mtile))
                                nc.vector.tensor_add(yo[:], t_[:], y1s[mtile][:])
                            nc.sync.dma_start(
                                out=out[mtile * 128:(mtile + 1) * 128, osl],
                                in_=yo[:])

                def emit_ag(j):
                    with nc.named_scope(f"ag{j}"):
                        nc.gpsimd.collective_compute(
                            "AllGather", OP.bypass,
                            replica_groups=[list(range(NCORE))],
                            ins=[sends[j].opt()], outs=[recvs[j].opt()],
                        )

                emit_attn_half(0)
                emit_ag(0)
                emit_attn_half(1)
                emit_c_half(0)   # scheduler back-fills attn1's PE gaps
                emit_ag(1)
                emit_c_half(1)
            ctx_vb.__exit__(None, None, None)
            ctx_qv.__exit__(None, None, None)

    nc.finalize()
    return nc


def _prep_inputs(x, g1, b1, Wq, bq, Wk, bk, Wv, bv, Wo, bo, g2, b2, W1, bf1,
                 W2, bf2):
    bf = ml_dtypes.bfloat16
    f32 = np.float32

    def tile4(A, n_m):
        # A is [C, n_m*128] (already transposed): -> [n_m, 128, KC, 128]
        return np.ascontiguousarray(
            A.reshape(KC, 128, n_m, 128).transpose(2, 1, 0, 3)).astype(bf)

    w1t_full = np.ascontiguousarray(
        W1.T.reshape(KC, 128, FT, 128).transpose(2, 1, 0, 3)).astype(bf)
    w2t_full = np.ascontiguousarray(
        W2.T.reshape(FT, 128, CT, 128).transpose(2, 1, 0, 3)).astype(bf)
    wot_full = tile4(Wo.T, CT)

    trivial = (
        np.all(g1 == 1) and np.all(g2 == 1)
        and not (np.any(b1) or np.any(b2) or np.any(bq) or np.any(bk)
                 or np.any(bv) or np.any(bo) or np.any(bf1) or np.any(bf2))
    )

    in_maps = []
    for c in range(NCORE):
        b, r = divmod(c, R)
        hd0 = HDL * r
        xT = np.ascontiguousarray(x[b].T)

        pack = np.zeros((128, N_BIAS_COLS), f32)
        for j in range(2):
            pack[:, COL_BQ + j] = bq[hd0 + 128 * j: hd0 + 128 * (j + 1)]
            pack[:, COL_BK + j] = bk[hd0 + 128 * j: hd0 + 128 * (j + 1)]
        for j in range(CT):
            pack[:, COL_BO + j] = bo[128 * j: 128 * (j + 1)]
            pack[:, COL_G1 + j] = g1[128 * j: 128 * (j + 1)]
            pack[:, COL_B1 + j] = b1[128 * j: 128 * (j + 1)]
            pack[:, COL_G2 + j] = g2[128 * j: 128 * (j + 1)]
            pack[:, COL_B2 + j] = b2[128 * j: 128 * (j + 1)]
            pack[:, COL_BF2 + j] = bf2[128 * j: 128 * (j + 1)]
        for j in range(FT):
            pack[:, COL_BF1 + j] = bf1[128 * j: 128 * (j + 1)]

        # residual slab: this core owns tokens [1024*j + 256*r, +256) for j=0,1
        TH = T // 2
        xres_core = np.concatenate(
            [xT[:, 1024 * j + TH * r: 1024 * j + TH * (r + 1)] for j in range(2)],
            axis=1)
        in_maps.append({
            "xt": xT.astype(bf),
            "xres": np.ascontiguousarray(xres_core).astype(f32),
            "wqt": tile4(np.ascontiguousarray(Wq[hd0:hd0 + HDL, :].T), 2),
            "wkt": tile4(np.ascontiguousarray(Wk[hd0:hd0 + HDL, :].T), 2),
            "wvt": np.ascontiguousarray(
                Wv[hd0:hd0 + HDL, :].T.reshape(KC, 128, HDL)
                .transpose(1, 0, 2)).astype(bf),
            "wot": wot_full,
            "w1t": w1t_full,
            "w2t": w2t_full,
            "biasp": pack,
            "bvrow": np.ascontiguousarray(bv[hd0:hd0 + HDL]).reshape(1, HDL)
                .astype(f32),
        })
    return in_maps, trivial


def _run(in_maps, trivial=True, trace=False, trace_cores=None):
    key = f"nc_{trivial}"
    if key not in _prog_cache:
        _prog_cache[key] = build_program(trivial=trivial)
    nc = _prog_cache[key]
    return run_bass_kernel_spmd(
        nc, in_maps, core_ids=list(range(NCORE)), trace=trace,
        trace_cores=trace_cores)


def assemble_output(results):
    TH = T // 2
    out_full = np.empty((B, L, C), np.float32)
    for c in range(NCORE):
        b, r = divmod(c, R)
        y = results[c]["out"]
        for j in range(2):
            out_full[b, 1024 * j + TH * r: 1024 * j + TH * (r + 1), :] = \
                y[:, TH * j: TH * (j + 1)].T
    return out_full


def kernel(**inputs):
    in_maps, trivial = _prep_inputs(
        **{k: np.asarray(v) for k, v in inputs.items()})
    res = _run(in_maps, trivial=trivial)
    return assemble_output(res.results)
